# revision 2
# baseline (speedup 1.0000x reference)
"""MoE SwiGLU (T=4096, D=I=1024, E=8, top-2) on 8 Trainium2 NeuronCores.

Expert-parallel: core e holds expert e's weights (w1/w3/w2) in SBUF and
computes its expert's SwiGLU over all tokens, scaled by that expert's
routing weight (softmax prob if expert in token's top-2, else 0).  The
gate (scores -> softmax -> top-2 mask) is replicated on every core and
computed in true fp32 for selection fidelity; the heavy matmuls run as
float32r (FP22) at full PE rate.  Per-core contributions are combined
with an on-device ReduceScatter; each core returns its 512-token shard
of the output and the host concatenates.
"""
import os
import sys

import numpy as np

for _p in ("/opt/trn_rl_repo", "/root/.axon_site/_ro/trn_rl_repo"):
    if os.path.isdir(_p) and _p not in sys.path:
        sys.path.append(_p)

import concourse.bass as bass  # noqa: E402
import concourse.mybir as mybir  # noqa: E402
import concourse.tile as tile  # noqa: E402
from concourse import bacc  # noqa: E402
from concourse.bass_utils import run_bass_kernel_spmd  # noqa: E402

P = 128
T, D, I, E, TOPK = 4096, 1024, 1024, 8, 2
NCORES = 8
TCH = 512            # token chunk (matmul free dim)
NCH = T // TCH       # 8 chunks
TSH = T // NCORES    # 512 tokens per output shard
DK = D // P          # 8 contraction tiles
IK = I // P          # 8
f32 = mybir.dt.float32
f32r = mybir.dt.float32r

_CACHED_NC = None


def _build():
    nc = bacc.Bacc("TRN2", target_bir_lowering=False, debug=False,
                   num_devices=NCORES)
    xT_d = nc.dram_tensor("xT", [D, T], f32, kind="ExternalInput")
    gwT_d = nc.dram_tensor("gwT", [D, E], f32, kind="ExternalInput")
    w1T_d = nc.dram_tensor("w1T", [D, I], f32r, kind="ExternalInput")
    w3T_d = nc.dram_tensor("w3T", [D, I], f32r, kind="ExternalInput")
    w2T_d = nc.dram_tensor("w2T", [I, D], f32r, kind="ExternalInput")
    y_d = nc.dram_tensor("y", [TSH, D], f32, kind="ExternalOutput")

    with tile.TileContext(nc) as tc:
        with tc.tile_pool(name="wpool", bufs=1) as wpool, \
             tc.tile_pool(name="xpool", bufs=2) as xpool, \
             tc.tile_pool(name="gxpool", bufs=1) as gxpool, \
             tc.tile_pool(name="apool", bufs=2) as apool, \
             tc.tile_pool(name="spool", bufs=2) as spool, \
             tc.tile_pool(name="gpool", bufs=2) as gpool, \
             tc.tile_pool(name="ypool", bufs=3) as ypool, \
             tc.tile_pool(name="psum", bufs=2, space="PSUM") as psum, \
             tc.tile_pool(name="gpsum", bufs=2, space="PSUM") as gpsum, \
             tc.tile_pool(name="dram", bufs=1, space="DRAM") as dram:

            # --- resident weights ---
            w1T_s = wpool.tile([P, DK, I], f32r, tag="w1")
            nc.sync.dma_start(w1T_s[:], w1T_d[:, :].rearrange("(o p) i -> p o i", p=P))
            w3T_s = wpool.tile([P, DK, I], f32r, tag="w3")
            nc.sync.dma_start(w3T_s[:], w3T_d[:, :].rearrange("(o p) i -> p o i", p=P))
            w2T_s = wpool.tile([P, IK, D], f32r, tag="w2")
            nc.sync.dma_start(w2T_s[:], w2T_d[:, :].rearrange("(o p) d -> p o d", p=P))
            gwT_s = wpool.tile([P, DK, E], f32, tag="gw")
            nc.sync.dma_start(gwT_s[:], gwT_d[:, :].rearrange("(o p) e -> p o e", p=P))

            ycontrib = dram.tile([T, D], f32)
            yshard = dram.tile([TSH, D], f32)

            for t in range(NCH):
                tsl = slice(t * TCH, (t + 1) * TCH)
                # x^T chunk: fp32 copy (gate) + f32r copy (main matmuls)
                xg_s = gxpool.tile([P, DK, TCH], f32, tag="xg")
                nc.sync.dma_start(
                    xg_s[:], xT_d[:, tsl].rearrange("(o p) t -> p o t", p=P))
                xr_s = xpool.tile([P, DK, TCH], f32r, tag="xr")
                nc.gpsimd.dma_start(
                    xr_s[:], xT_d[:, tsl].rearrange("(o p) t -> p o t", p=P))

                # --- gate: scores -> softmax -> top-2 weight for expert 0 ---
                # (host permutes gate columns so this core's expert is col 0)
                wgt = gpool.tile([P, 4], f32, tag="wgt")  # per t_tile scalars
                for tt in range(4):
                    ps_g = gpsum.tile([P, E], f32, tag="gps")
                    for dk in range(DK):
                        nc.tensor.matmul(
                            ps_g[:],
                            lhsT=xg_s[:, dk, tt * P:(tt + 1) * P],
                            rhs=gwT_s[:, dk, :],
                            start=(dk == 0), stop=(dk == DK - 1))
                    negmx = gpool.tile([P, 1], f32, tag="negmx")
                    nc.vector.tensor_reduce(
                        negmx[:], ps_g[:], mybir.AxisListType.X,
                        mybir.AluOpType.max)
                    nc.vector.tensor_scalar_mul(negmx[:], negmx[:], -1.0)
                    probs = gpool.tile([P, E], f32, tag="probs")
                    sumexp = gpool.tile([P, 1], f32, tag="sumexp")
                    nc.scalar.activation(
                        probs[:], ps_g[:], mybir.ActivationFunctionType.Exp,
                        bias=negmx[:, 0:1], accum_out=sumexp[:, 0:1])
                    recip = gpool.tile([P, 1], f32, tag="recip")
                    nc.vector.reciprocal(recip[:], sumexp[:])
                    nc.vector.tensor_scalar_mul(probs[:], probs[:], recip[:, 0:1])
                    mx8 = gpool.tile([P, 8], f32, tag="mx8")
                    nc.vector.max(mx8[:], probs[:])
                    ge = gpool.tile([P, 1], f32, tag="ge")
                    nc.vector.tensor_tensor(
                        ge[:], probs[:, 0:1], mx8[:, 1:2], mybir.AluOpType.is_ge)
                    nc.vector.tensor_mul(
                        wgt[:, tt:tt + 1], probs[:, 0:1], ge[:])

                # --- a^T = silu(w1 x^T) * (w3 x^T), laid out [I, tokens] ---
                aT_s = apool.tile([P, IK, TCH], f32r, tag="aT")
                for ik in range(IK):
                    isl = slice(ik * P, (ik + 1) * P)
                    ph = psum.tile([P, TCH], f32, tag="ph")
                    for dk in range(DK):
                        nc.tensor.matmul(
                            ph[:], lhsT=w1T_s[:, dk, isl], rhs=xr_s[:, dk, :],
                            start=(dk == 0), stop=(dk == DK - 1))
                    pg = psum.tile([P, TCH], f32, tag="pg")
                    for dk in range(DK):
                        nc.tensor.matmul(
                            pg[:], lhsT=w3T_s[:, dk, isl], rhs=xr_s[:, dk, :],
                            start=(dk == 0), stop=(dk == DK - 1))
                    sil = spool.tile([P, TCH], f32r, tag="sil")
                    nc.scalar.activation(
                        sil[:], ph[:], mybir.ActivationFunctionType.Silu)
                    nc.vector.tensor_mul(aT_s[:, ik, :], sil[:], pg[:])

                # --- y chunk = (a^T)^T @ w2^T, scaled by routing weight ---
                for tt in range(4):
                    for dc in range(2):
                        py = psum.tile([P, TCH], f32, tag="py")
                        for ik in range(IK):
                            nc.tensor.matmul(
                                py[:],
                                lhsT=aT_s[:, ik, tt * P:(tt + 1) * P],
                                rhs=w2T_s[:, ik, dc * TCH:(dc + 1) * TCH],
                                start=(ik == 0), stop=(ik == IK - 1))
                        yt = ypool.tile([P, TCH], f32, tag="yt")
                        nc.vector.tensor_scalar_mul(
                            yt[:], py[:], wgt[:, tt:tt + 1])
                        r0 = t * TCH + tt * P
                        nc.sync.dma_start(
                            ycontrib[r0:r0 + P, dc * TCH:(dc + 1) * TCH], yt[:])

            nc.gpsimd.collective_compute(
                "ReduceScatter",
                mybir.AluOpType.add,
                replica_groups=[list(range(NCORES))],
                ins=[ycontrib.opt()],
                outs=[yshard.opt()],
            )
            nc.sync.dma_start(y_d[:, :], yshard[:])
    nc.compile()
    return nc


def _get_nc():
    global _CACHED_NC
    if _CACHED_NC is None:
        _CACHED_NC = _build()
    return _CACHED_NC


def _in_maps(x, gate_w, w1, w3, w2):
    xT = np.ascontiguousarray(np.asarray(x, dtype=np.float32).T)
    maps = []
    for e in range(NCORES):
        perm = [e] + [j for j in range(E) if j != e]
        gwT = np.ascontiguousarray(
            np.asarray(gate_w, dtype=np.float32)[perm].T)   # [D, E], col0 = e
        maps.append({
            "xT": xT,
            "gwT": gwT,
            "w1T": np.ascontiguousarray(np.asarray(w1[e], np.float32).T),
            "w3T": np.ascontiguousarray(np.asarray(w3[e], np.float32).T),
            "w2T": np.ascontiguousarray(np.asarray(w2[e], np.float32).T),
        })
    return maps


def run(x, gate_w, w1, w3, w2, trace=False, trace_cores=None):
    nc = _get_nc()
    maps = _in_maps(x, gate_w, w1, w3, w2)
    res = run_bass_kernel_spmd(
        nc, maps, core_ids=list(range(NCORES)), trace=trace,
        trace_cores=trace_cores)
    y = np.concatenate([res.results[r]["y"] for r in range(NCORES)], axis=0)
    return y, res


def kernel(x, gate_w, w1, w3, w2):
    y, _ = run(x, gate_w, w1, w3, w2, trace=False)
    return y.astype(np.float32)


# revision 6
# speedup vs baseline: 1.1683x; 1.1683x over previous
"""MoE SwiGLU (T=4096, D=I=1024, E=8, top-2) on 8 Trainium2 NeuronCores.

Expert-parallel: core e holds expert e's weights (w1/w3/w2) in SBUF and
computes its expert's SwiGLU over all tokens, scaled by that expert's
routing weight (softmax prob if expert in token's top-2, else 0).  The
gate (scores -> softmax -> top-2 mask) is replicated on every core and
computed in true fp32 for selection fidelity; the heavy matmuls run as
float32r (FP22) at full PE rate.  Per-core contributions are combined
with an on-device ReduceScatter; each core returns its 512-token shard
of the output and the host concatenates.
"""
import os
import sys

import numpy as np

for _p in ("/opt/trn_rl_repo", "/root/.axon_site/_ro/trn_rl_repo"):
    if os.path.isdir(_p) and _p not in sys.path:
        sys.path.append(_p)

import concourse.bass as bass  # noqa: E402
import concourse.mybir as mybir  # noqa: E402
import concourse.tile as tile  # noqa: E402
from concourse import bacc  # noqa: E402
from concourse.bass_utils import run_bass_kernel_spmd  # noqa: E402

P = 128
T, D, I, E, TOPK = 4096, 1024, 1024, 8, 2
NCORES = 8
TCH = 512            # token chunk (matmul free dim)
NCH = T // TCH       # 8 chunks
TSH = T // NCORES    # 512 tokens per output shard
DK = D // P          # 8 contraction tiles
IK = I // P          # 8
f32 = mybir.dt.float32
f32r = mybir.dt.float32r

_CACHED_NC = None


def _build():
    nc = bacc.Bacc("TRN2", target_bir_lowering=False, debug=False,
                   num_devices=NCORES)
    xT_d = nc.dram_tensor("xT", [D, T], f32, kind="ExternalInput")
    gwT_d = nc.dram_tensor("gwT", [D, E], f32, kind="ExternalInput")
    w1T_d = nc.dram_tensor("w1T", [D, I], f32r, kind="ExternalInput")
    w3T_d = nc.dram_tensor("w3T", [D, I], f32r, kind="ExternalInput")
    w2T_d = nc.dram_tensor("w2T", [I, D], f32r, kind="ExternalInput")
    y_d = nc.dram_tensor("y", [TSH, D], f32, kind="ExternalOutput")

    with tile.TileContext(nc) as tc:
        with tc.tile_pool(name="wpool", bufs=1) as wpool, \
             tc.tile_pool(name="xpool", bufs=2) as xpool, \
             tc.tile_pool(name="gxpool", bufs=1) as gxpool, \
             tc.tile_pool(name="apool", bufs=2) as apool, \
             tc.tile_pool(name="spool", bufs=2) as spool, \
             tc.tile_pool(name="gpool", bufs=2) as gpool, \
             tc.tile_pool(name="ypool", bufs=3) as ypool, \
             tc.tile_pool(name="psum", bufs=2, space="PSUM") as psum, \
             tc.tile_pool(name="gpsum", bufs=2, space="PSUM") as gpsum, \
             tc.tile_pool(name="dram", bufs=1, space="DRAM") as dram:

            # --- resident weights (split loads so first matmuls start early) ---
            gwT_s = wpool.tile([P, DK, E], f32, tag="gw")
            nc.sync.dma_start(gwT_s[:], gwT_d[:, :].rearrange("(o p) e -> p o e", p=P))
            w1T_s = wpool.tile([P, DK, I], f32r, tag="w1")
            w3T_s = wpool.tile([P, DK, I], f32r, tag="w3")
            w2T_s = wpool.tile([P, IK, D], f32r, tag="w2")
            for h in range(4):
                hs = slice(h * (I // 4), (h + 1) * (I // 4))
                nc.sync.dma_start(
                    w1T_s[:, :, hs],
                    w1T_d[:, hs].rearrange("(o p) i -> p o i", p=P))
                nc.sync.dma_start(
                    w3T_s[:, :, hs],
                    w3T_d[:, hs].rearrange("(o p) i -> p o i", p=P))
                nc.sync.dma_start(
                    w2T_s[:, :, hs],
                    w2T_d[:, hs].rearrange("(o p) d -> p o d", p=P))

            # 4 token-range pieces so ReduceScatter overlaps compute
            NRS = 4
            RT = T // NRS          # 1024 tokens per RS piece
            RSH = RT // NCORES     # 128-token shard per core per piece
            ycontribs = [dram.tile([RT, D], f32, tag=f"yc{q}", name=f"yc{q}") for q in range(NRS)]
            yshards = [dram.tile([RSH, D], f32, tag=f"ys{q}", name=f"ys{q}") for q in range(NRS)]

            for t in range(NCH):
                tsl = slice(t * TCH, (t + 1) * TCH)
                # x^T chunk: fp32 copy (gate) + f32r copy (main matmuls)
                xg_s = gxpool.tile([P, DK, TCH], f32, tag="xg")
                nc.sync.dma_start(
                    xg_s[:], xT_d[:, tsl].rearrange("(o p) t -> p o t", p=P))
                xr_s = xpool.tile([P, DK, TCH], f32r, tag="xr")
                nc.gpsimd.dma_start(
                    xr_s[:], xT_d[:, tsl].rearrange("(o p) t -> p o t", p=P))

                # --- gate: scores -> softmax -> top-2 weight for expert 0 ---
                # (host permutes gate columns so this core's expert is col 0)
                wgt = gpool.tile([P, 4], f32, tag="wgt")  # per t_tile scalars
                for tt in range(4):
                    ps_g = gpsum.tile([P, E], f32, tag="gps")
                    for dk in range(DK):
                        nc.tensor.matmul(
                            ps_g[:],
                            lhsT=xg_s[:, dk, tt * P:(tt + 1) * P],
                            rhs=gwT_s[:, dk, :],
                            start=(dk == 0), stop=(dk == DK - 1))
                    negmx = gpool.tile([P, 1], f32, tag="negmx")
                    nc.vector.tensor_reduce(
                        negmx[:], ps_g[:], mybir.AxisListType.X,
                        mybir.AluOpType.max)
                    nc.vector.tensor_scalar_mul(negmx[:], negmx[:], -1.0)
                    probs = gpool.tile([P, E], f32, tag="probs")
                    sumexp = gpool.tile([P, 1], f32, tag="sumexp")
                    nc.scalar.activation(
                        probs[:], ps_g[:], mybir.ActivationFunctionType.Exp,
                        bias=negmx[:, 0:1], accum_out=sumexp[:, 0:1])
                    recip = gpool.tile([P, 1], f32, tag="recip")
                    nc.vector.reciprocal(recip[:], sumexp[:])
                    nc.vector.tensor_scalar_mul(probs[:], probs[:], recip[:, 0:1])
                    mx8 = gpool.tile([P, 8], f32, tag="mx8")
                    nc.vector.max(mx8[:], probs[:])
                    ge = gpool.tile([P, 1], f32, tag="ge")
                    nc.vector.tensor_tensor(
                        ge[:], probs[:, 0:1], mx8[:, 1:2], mybir.AluOpType.is_ge)
                    nc.vector.tensor_mul(
                        wgt[:, tt:tt + 1], probs[:, 0:1], ge[:])

                # --- a^T = silu(w1 x^T) * (w3 x^T), laid out [I, tokens] ---
                aT_s = apool.tile([P, IK, TCH], f32r, tag="aT")
                for ik in range(IK):
                    isl = slice(ik * P, (ik + 1) * P)
                    ph = psum.tile([P, TCH], f32, tag="ph")
                    for dk in range(DK):
                        nc.tensor.matmul(
                            ph[:], lhsT=w1T_s[:, dk, isl], rhs=xr_s[:, dk, :],
                            start=(dk == 0), stop=(dk == DK - 1))
                    pg = psum.tile([P, TCH], f32, tag="pg")
                    for dk in range(DK):
                        nc.tensor.matmul(
                            pg[:], lhsT=w3T_s[:, dk, isl], rhs=xr_s[:, dk, :],
                            start=(dk == 0), stop=(dk == DK - 1))
                    sil = spool.tile([P, TCH], f32r, tag="sil")
                    nc.scalar.activation(
                        sil[:], ph[:], mybir.ActivationFunctionType.Silu)
                    nc.vector.tensor_mul(aT_s[:, ik, :], sil[:], pg[:])

                # --- y chunk = (a^T)^T @ w2^T, scaled by routing weight ---
                for tt in range(4):
                    for dc in range(2):
                        py = psum.tile([P, TCH], f32, tag="py")
                        for ik in range(IK):
                            nc.tensor.matmul(
                                py[:],
                                lhsT=aT_s[:, ik, tt * P:(tt + 1) * P],
                                rhs=w2T_s[:, ik, dc * TCH:(dc + 1) * TCH],
                                start=(ik == 0), stop=(ik == IK - 1))
                        yt = ypool.tile([P, TCH], f32, tag="yt")
                        nc.vector.tensor_scalar_mul(
                            yt[:], py[:], wgt[:, tt:tt + 1])
                        r0 = t * TCH + tt * P
                        q, qr = divmod(r0, RT)
                        nc.sync.dma_start(
                            ycontribs[q][qr:qr + P, dc * TCH:(dc + 1) * TCH],
                            yt[:])

                if t % 2 == 1:
                    q = t // 2
                    nc.gpsimd.collective_compute(
                        "ReduceScatter",
                        mybir.AluOpType.add,
                        replica_groups=[list(range(NCORES))],
                        ins=[ycontribs[q].opt()],
                        outs=[yshards[q].opt()],
                    )
                    nc.sync.dma_start(
                        y_d[q * RSH:(q + 1) * RSH, :], yshards[q][:])
    nc.compile()
    return nc


def _get_nc():
    global _CACHED_NC
    if _CACHED_NC is None:
        _CACHED_NC = _build()
    return _CACHED_NC


def _in_maps(x, gate_w, w1, w3, w2):
    xT = np.ascontiguousarray(np.asarray(x, dtype=np.float32).T)
    maps = []
    for e in range(NCORES):
        perm = [e] + [j for j in range(E) if j != e]
        gwT = np.ascontiguousarray(
            np.asarray(gate_w, dtype=np.float32)[perm].T)   # [D, E], col0 = e
        maps.append({
            "xT": xT,
            "gwT": gwT,
            "w1T": np.ascontiguousarray(np.asarray(w1[e], np.float32).T),
            "w3T": np.ascontiguousarray(np.asarray(w3[e], np.float32).T),
            "w2T": np.ascontiguousarray(np.asarray(w2[e], np.float32).T),
        })
    return maps


def run(x, gate_w, w1, w3, w2, trace=False, trace_cores=None):
    nc = _get_nc()
    maps = _in_maps(x, gate_w, w1, w3, w2)
    res = run_bass_kernel_spmd(
        nc, maps, core_ids=list(range(NCORES)), trace=trace,
        trace_cores=trace_cores)
    # core r's output block q (128 rows) holds tokens [1024q + 128r, +128)
    y = np.empty((T, D), dtype=np.float32)
    for r in range(NCORES):
        yr = res.results[r]["y"]
        for q in range(4):
            t0 = q * 1024 + r * P
            y[t0:t0 + P] = yr[q * P:(q + 1) * P]
    return y, res


def kernel(x, gate_w, w1, w3, w2):
    y, _ = run(x, gate_w, w1, w3, w2, trace=False)
    return y.astype(np.float32)
